# revision 2
# baseline (speedup 1.0000x reference)
"""Trainium2 Bass kernel for nn_Attn_Pred_Model (sparse_attention).

Math (per batch b, channel c):
    decay[t] = sum_{i=0}^{P-1} alpha * beta**i * x[t-i-1]        (P = past_steps)
    out[s,c] = (decay + pos_bias_fwd[c] + pos_bias_bwd[arange2[s,c]]) * mask[s,c]

Mapping:
  The causal exponential conv along S is a banded lower-triangular matmul.
  We put S on the contraction/partition axis and (batch, channel) on the
  moving free axis, processing S in 128-row chunks:
      out_chunk = Wprev.T @ x_prev_chunk[64:128] + Wdiag.T @ x_chunk
  Both weight matrices are constant across chunks and batches.

  With S = NB*NB and bucket stride NB, arange2 and mask are constant within
  64-row s-blocks, so the gathered bias reduces to one 64-value row per
  (block, channel), added via a rank-1 K=2 matmul into the same PSUM
  accumulation, and the causal mask reduces to a per-(chunk, channel)
  multiplier applied during the PSUM->SBUF copy.

Sharding: data-parallel over the batch dim across 8 cores (16 batches each).
Host side only reshuffles data layout (B,S,C)->(S,B,C) and builds the tiny
(<=128KB) weight/bias/mask tables; all O(B*S*C) compute runs on device.
"""

import numpy as np
from contextlib import ExitStack

import concourse.bass as bass
import concourse.tile as tile
from concourse import bacc, mybir
from concourse.bass import ts
from concourse.bass_utils import run_bass_kernel_spmd

N_CORES = 8
NB = 64            # channels / num buckets
CHUNK = 128        # s-rows per chunk (PE contraction tile)
BG = 8             # batches per matmul free-group (8*NB = 512 = fp32 PSUM bank)
NFREE = BG * NB    # 512


# ---------------------------------------------------------------- device code

def _kernel_body(ctx, tc, aps, S, BL, repeats, mm_dt):
    nc = tc.nc
    nchunk = S // CHUNK
    nt = BL // BG  # free-groups per chunk

    consts = ctx.enter_context(tc.tile_pool(name="consts", bufs=1))
    xpool = ctx.enter_context(tc.tile_pool(name="xchunks", bufs=3))
    opool = ctx.enter_context(tc.tile_pool(name="outs", bufs=4))
    ppool = ctx.enter_context(tc.tile_pool(name="psum", bufs=4, space="PSUM"))

    f32 = mybir.dt.float32

    wdiag_sb = consts.tile([128, 128], f32)
    nc.sync.dma_start(wdiag_sb[:], aps["wdiag"])
    wprev_sb = consts.tile([128, 128], f32)
    nc.sync.dma_start(wprev_sb[:], aps["wprev"])
    wbias_sb = consts.tile([2, 128], f32)
    nc.sync.dma_start(wbias_sb[:], aps["wbias"])
    biasrows_sb = consts.tile([2, nchunk * NFREE], f32)
    nc.sync.dma_start(biasrows_sb[:], aps["biasrows"])
    maskt_sb = consts.tile([128, nchunk * NB], f32)
    nc.sync.dma_start(maskt_sb[:], aps["maskt"])

    x_ap = aps["x"]    # (S, BL, NB)
    y_ap = aps["y"]    # (S, BL, NB)

    def one_pass():
        prev = None
        for t in range(nchunk):
            xt = xpool.tile([128, BL * NB], f32, tag="x")
            nc.sync.dma_start(
                xt[:].rearrange("p (b c) -> p b c", c=NB),
                x_ap[t * CHUNK:(t + 1) * CHUNK],
            )
            for g in range(nt):
                ps = ppool.tile([128, NFREE], f32, tag="ps")
                if prev is not None:
                    nc.tensor.matmul(
                        ps[:],
                        wprev_sb[64:128, :].bitcast(mm_dt),
                        prev[64:128, ts(g, NFREE)].bitcast(mm_dt),
                        start=True, stop=False,
                    )
                nc.tensor.matmul(
                    ps[:],
                    wdiag_sb[:].bitcast(mm_dt),
                    xt[:, ts(g, NFREE)].bitcast(mm_dt),
                    start=(prev is None), stop=False,
                )
                nc.tensor.matmul(
                    ps[:],
                    wbias_sb[:].bitcast(mm_dt),
                    biasrows_sb[:, ts(t, NFREE)].bitcast(mm_dt),
                    start=False, stop=True,
                )
                ot = opool.tile([128, NFREE], f32, tag="o")
                m3 = (maskt_sb[:, ts(t, NB)]
                      .rearrange("p (one c) -> p one c", one=1)
                      .broadcast_to((128, BG, NB)))
                nc.vector.tensor_mul(
                    ot[:].rearrange("p (b c) -> p b c", c=NB),
                    ps[:].rearrange("p (b c) -> p b c", c=NB),
                    m3,
                )
                nc.scalar.dma_start(
                    y_ap[t * CHUNK:(t + 1) * CHUNK, g * BG:(g + 1) * BG],
                    ot[:].rearrange("p (b c) -> p b c", c=NB),
                )
            prev = xt

    if repeats == 1:
        one_pass()
    else:
        with tc.For_i(0, repeats, 1):
            one_pass()


_NC_CACHE = {}


def _build_nc(S, BL, repeats, mm_dt_name):
    key = (S, BL, repeats, mm_dt_name)
    if key in _NC_CACHE:
        return _NC_CACHE[key]
    mm_dt = getattr(mybir.dt, mm_dt_name)
    f32 = mybir.dt.float32
    nchunk = S // CHUNK
    nc = bacc.Bacc("TRN2", target_bir_lowering=False, debug=False)
    aps = {
        "x": nc.dram_tensor("x", (S, BL, NB), f32, kind="ExternalInput").ap(),
        "wdiag": nc.dram_tensor("wdiag", (128, 128), f32, kind="ExternalInput").ap(),
        "wprev": nc.dram_tensor("wprev", (128, 128), f32, kind="ExternalInput").ap(),
        "wbias": nc.dram_tensor("wbias", (2, 128), f32, kind="ExternalInput").ap(),
        "biasrows": nc.dram_tensor(
            "biasrows", (2, nchunk * NFREE), f32, kind="ExternalInput").ap(),
        "maskt": nc.dram_tensor(
            "maskt", (128, nchunk * NB), f32, kind="ExternalInput").ap(),
        "y": nc.dram_tensor("y", (S, BL, NB), f32, kind="ExternalOutput").ap(),
    }
    with tile.TileContext(nc) as tc:
        with ExitStack() as ctx:
            _kernel_body(ctx, tc, aps, S, BL, repeats, mm_dt)
    nc.compile()
    _NC_CACHE[key] = nc
    return nc


# ------------------------------------------------------------------ host prep

def _coeff(alpha, beta, past_steps):
    """coeff[d-1] = weight of x[t-d] in decay[t], d = 1..64."""
    d = np.arange(1, 65, dtype=np.float64)
    c = np.where(d <= past_steps, float(alpha) * float(beta) ** (d - 1), 0.0)
    return c.astype(np.float32)


def _weights(alpha, beta, past_steps):
    c = np.zeros(256, dtype=np.float32)
    c[1:65] = _coeff(alpha, beta, past_steps)

    k = np.arange(128)[:, None]
    m = np.arange(128)[None, :]
    d_diag = m - k          # s_out=(r0+m), s_in=(r0+k)
    d_prev = m + 128 - k    # s_in = r0-128+k
    wdiag = np.where((d_diag >= 1) & (d_diag <= 64), c[np.clip(d_diag, 0, 255)], 0.0)
    wprev = np.where((d_prev >= 1) & (d_prev <= 64), c[np.clip(d_prev, 0, 255)], 0.0)

    wbias = np.zeros((2, 128), dtype=np.float32)
    wbias[0, :64] = 1.0
    wbias[1, 64:] = 1.0
    return wdiag.astype(np.float32), wprev.astype(np.float32), wbias


def _tables(pos_bias_fwd, pos_bias_bwd, arange2, mask, S):
    """biasrows (2, nchunk*NFREE) and maskt (128, nchunk*NB) from the inputs.

    Relies on arange2 being constant within each 64-row s-block (structural:
    arange2[s,c] = ((s - c*NB) % S)//NB and blocks are s//64)."""
    nchunk = S // CHUNK
    nblk = S // 64
    a2 = np.asarray(arange2)
    blk = a2.reshape(nblk, 64, NB)
    assert (blk == blk[:, :1, :]).all(), "arange2 not block-constant"

    B = np.asarray(pos_bias_fwd)[0][None, :] + np.asarray(pos_bias_bwd)[0][blk[:, 0, :]]
    # biasrows[p, t, b, c] = B[2t+p, c]
    Bp = B.reshape(nchunk, 2, NB).transpose(1, 0, 2)        # (2, nchunk, NB)
    biasrows = np.broadcast_to(
        Bp[:, :, None, :], (2, nchunk, BG, NB)).reshape(2, nchunk * NFREE)

    mk = np.asarray(mask, dtype=np.float32).reshape(nchunk, CHUNK, NB)
    maskt = mk.transpose(1, 0, 2).reshape(CHUNK, nchunk * NB)
    return np.ascontiguousarray(biasrows, dtype=np.float32), \
        np.ascontiguousarray(maskt)


def _make_in_maps(x, pos_bias_fwd, pos_bias_bwd, beta, alpha, arange2, mask,
                  past_steps, n_cores=N_CORES):
    B, S, C = x.shape
    assert C == NB and S % CHUNK == 0 and B % (n_cores * BG) == 0
    BL = B // n_cores
    P = int(np.asarray(past_steps))
    assert 1 <= P <= 64, f"past_steps={P} outside supported window"

    wdiag, wprev, wbias = _weights(np.asarray(alpha)[0], np.asarray(beta)[0], P)
    biasrows, maskt = _tables(pos_bias_fwd, pos_bias_bwd, arange2, mask, S)

    common = {
        "wdiag": wdiag, "wprev": wprev, "wbias": wbias,
        "biasrows": biasrows, "maskt": maskt,
    }
    in_maps = []
    for i in range(n_cores):
        xs = np.ascontiguousarray(
            x[i * BL:(i + 1) * BL].transpose(1, 0, 2))     # (S, BL, NB)
        in_maps.append({"x": xs, **common})
    return in_maps, BL


def _run(x, pos_bias_fwd, pos_bias_bwd, beta, alpha, arange2, mask, past_steps,
         repeats=1, mm_dt_name="float32r"):
    B, S, C = x.shape
    in_maps, BL = _make_in_maps(
        x, pos_bias_fwd, pos_bias_bwd, beta, alpha, arange2, mask, past_steps)
    nc = _build_nc(S, BL, repeats, mm_dt_name)
    res = run_bass_kernel_spmd(nc, in_maps, core_ids=list(range(N_CORES)))
    out = np.empty((B, S, C), dtype=np.float32)
    for i in range(N_CORES):
        out[i * BL:(i + 1) * BL] = res.results[i]["y"].transpose(1, 0, 2)
    return out


def kernel(x, pos_bias_fwd, pos_bias_bwd, beta, alpha, arange2, mask,
           past_steps, **_unused):
    x = np.asarray(x, dtype=np.float32)
    return _run(x, pos_bias_fwd, pos_bias_bwd, beta, alpha, arange2, mask,
                past_steps)


# revision 5
# speedup vs baseline: 39.4132x; 39.4132x over previous
"""Trainium2 Bass kernel for nn_Attn_Pred_Model (sparse_attention).

Math (per batch b, channel c):
    decay[t] = sum_{i=0}^{P-1} alpha * beta**i * x[t-i-1]        (P = past_steps)
    out[s,c] = (decay + pos_bias_fwd[c] + pos_bias_bwd[arange2[s,c]]) * mask[s,c]

Mapping:
  The causal exponential conv along S is a banded lower-triangular matmul.
  We put S on the contraction/partition axis and (batch, channel) on the
  moving free axis, processing S in 128-row chunks:
      out_chunk = Wprev.T @ x_prev_chunk[64:128] + Wdiag.T @ x_chunk
  Both weight matrices are constant across chunks and batches.

  With S = NB*NB and bucket stride NB, arange2 and mask are constant within
  64-row s-blocks, so the gathered bias reduces to one 64-value row per
  (block, channel), added via a rank-1 K=2 matmul into the same PSUM
  accumulation, and the causal mask reduces to a per-(chunk, channel)
  multiplier applied during the PSUM->SBUF copy.

Sharding: data-parallel over the batch dim across 8 cores (16 batches each).
Host side only reshuffles data layout (B,S,C)->(S,B,C) and builds the tiny
(<=128KB) weight/bias/mask tables; all O(B*S*C) compute runs on device.
"""

import numpy as np
from contextlib import ExitStack

import concourse.bass as bass
import concourse.tile as tile
from concourse import bacc, mybir
from concourse.bass import ts
from concourse.bass_utils import run_bass_kernel_spmd

N_CORES = 8
NB = 64            # channels / num buckets
CHUNK = 128        # s-rows per chunk (PE contraction tile)
BG = 8             # batches per matmul free-group (8*NB = 512 = fp32 PSUM bank)
NFREE = BG * NB    # 512


# ---------------------------------------------------------------- device code

def _kernel_body(ctx, tc, aps, S, BL, repeats, mm_dt):
    nc = tc.nc
    nchunk = S // CHUNK
    nt = BL // BG  # free-groups per chunk

    consts = ctx.enter_context(tc.tile_pool(name="consts", bufs=1))
    xpool = ctx.enter_context(tc.tile_pool(name="xchunks", bufs=3))
    opool = ctx.enter_context(tc.tile_pool(name="outs", bufs=4))
    ppool = ctx.enter_context(tc.tile_pool(name="psum", bufs=4, space="PSUM"))

    f32 = mybir.dt.float32

    wdiag_sb = consts.tile([128, 128], mm_dt)
    nc.sync.dma_start(wdiag_sb[:], aps["wdiag"])
    wprev_sb = consts.tile([128, 128], mm_dt)
    nc.sync.dma_start(wprev_sb[:], aps["wprev"])
    wbias_sb = consts.tile([2, 128], mm_dt)
    nc.sync.dma_start(wbias_sb[:], aps["wbias"])
    biasrows_sb = consts.tile([2, nchunk * NFREE], mm_dt)
    nc.sync.dma_start(biasrows_sb[:], aps["biasrows"])
    maskt_sb = consts.tile([128, nchunk * NB], f32)
    nc.sync.dma_start(maskt_sb[:], aps["maskt"])

    x_ap = aps["x"]    # (S, BL, NB)
    y_ap = aps["y"]    # (S, BL, NB)

    def one_pass():
        prev = None
        for t in range(nchunk):
            xt = xpool.tile([128, BL * NB], mm_dt, tag="x")
            nc.sync.dma_start(
                xt[:].rearrange("p (b c) -> p b c", c=NB),
                x_ap[t * CHUNK:(t + 1) * CHUNK],
            )
            for g in range(nt):
                ps = ppool.tile([128, NFREE], f32, tag="ps")
                if prev is not None:
                    nc.tensor.matmul(
                        ps[:],
                        wprev_sb[64:128, :],
                        prev[64:128, ts(g, NFREE)],
                        start=True, stop=False,
                    )
                nc.tensor.matmul(
                    ps[:],
                    wdiag_sb[:],
                    xt[:, ts(g, NFREE)],
                    start=(prev is None), stop=False,
                )
                nc.tensor.matmul(
                    ps[:],
                    wbias_sb[:],
                    biasrows_sb[:, ts(t, NFREE)],
                    start=False, stop=True,
                )
                ot = opool.tile([128, NFREE], f32, tag="o")
                m3 = (maskt_sb[:, ts(t, NB)]
                      .rearrange("p (one c) -> p one c", one=1)
                      .broadcast_to((128, BG, NB)))
                nc.vector.tensor_mul(
                    ot[:].rearrange("p (b c) -> p b c", c=NB),
                    ps[:].rearrange("p (b c) -> p b c", c=NB),
                    m3,
                )
                nc.scalar.dma_start(
                    y_ap[t * CHUNK:(t + 1) * CHUNK, g * BG:(g + 1) * BG],
                    ot[:].rearrange("p (b c) -> p b c", c=NB),
                )
            prev = xt

    if repeats == 1:
        one_pass()
    else:
        with tc.For_i(0, repeats, 1):
            one_pass()


_NC_CACHE = {}


def _build_nc(S, BL, repeats, mm_dt_name):
    key = (S, BL, repeats, mm_dt_name)
    if key in _NC_CACHE:
        return _NC_CACHE[key]
    mm_dt = getattr(mybir.dt, mm_dt_name)
    f32 = mybir.dt.float32
    nchunk = S // CHUNK
    nc = bacc.Bacc("TRN2", target_bir_lowering=False, debug=False)
    aps = {
        "x": nc.dram_tensor("x", (S, BL, NB), mm_dt, kind="ExternalInput").ap(),
        "wdiag": nc.dram_tensor("wdiag", (128, 128), mm_dt,
                                kind="ExternalInput").ap(),
        "wprev": nc.dram_tensor("wprev", (128, 128), mm_dt,
                                kind="ExternalInput").ap(),
        "wbias": nc.dram_tensor("wbias", (2, 128), mm_dt,
                                kind="ExternalInput").ap(),
        "biasrows": nc.dram_tensor(
            "biasrows", (2, nchunk * NFREE), mm_dt, kind="ExternalInput").ap(),
        "maskt": nc.dram_tensor(
            "maskt", (128, nchunk * NB), f32, kind="ExternalInput").ap(),
        "y": nc.dram_tensor("y", (S, BL, NB), f32, kind="ExternalOutput").ap(),
    }
    with tile.TileContext(nc) as tc:
        with ExitStack() as ctx:
            _kernel_body(ctx, tc, aps, S, BL, repeats, mm_dt)
    nc.compile()
    _NC_CACHE[key] = nc
    return nc


# ------------------------------------------------------------------ host prep

def _coeff(alpha, beta, past_steps):
    """coeff[d-1] = weight of x[t-d] in decay[t], d = 1..64."""
    d = np.arange(1, 65, dtype=np.float64)
    c = np.where(d <= past_steps, float(alpha) * float(beta) ** (d - 1), 0.0)
    return c.astype(np.float32)


def _weights(alpha, beta, past_steps):
    c = np.zeros(256, dtype=np.float32)
    c[1:65] = _coeff(alpha, beta, past_steps)

    k = np.arange(128)[:, None]
    m = np.arange(128)[None, :]
    d_diag = m - k          # s_out=(r0+m), s_in=(r0+k)
    d_prev = m + 128 - k    # s_in = r0-128+k
    wdiag = np.where((d_diag >= 1) & (d_diag <= 64), c[np.clip(d_diag, 0, 255)], 0.0)
    wprev = np.where((d_prev >= 1) & (d_prev <= 64), c[np.clip(d_prev, 0, 255)], 0.0)

    wbias = np.zeros((2, 128), dtype=np.float32)
    wbias[0, :64] = 1.0
    wbias[1, 64:] = 1.0
    return wdiag.astype(np.float32), wprev.astype(np.float32), wbias


def _tables(pos_bias_fwd, pos_bias_bwd, arange2, mask, S):
    """biasrows (2, nchunk*NFREE) and maskt (128, nchunk*NB) from the inputs.

    Relies on arange2 being constant within each 64-row s-block (structural:
    arange2[s,c] = ((s - c*NB) % S)//NB and blocks are s//64)."""
    nchunk = S // CHUNK
    nblk = S // 64
    a2 = np.asarray(arange2)
    blk = a2.reshape(nblk, 64, NB)
    assert (blk == blk[:, :1, :]).all(), "arange2 not block-constant"

    B = np.asarray(pos_bias_fwd)[0][None, :] + np.asarray(pos_bias_bwd)[0][blk[:, 0, :]]
    # biasrows[p, t, b, c] = B[2t+p, c]
    Bp = B.reshape(nchunk, 2, NB).transpose(1, 0, 2)        # (2, nchunk, NB)
    biasrows = np.broadcast_to(
        Bp[:, :, None, :], (2, nchunk, BG, NB)).reshape(2, nchunk * NFREE)

    mk = np.asarray(mask, dtype=np.float32).reshape(nchunk, CHUNK, NB)
    maskt = mk.transpose(1, 0, 2).reshape(CHUNK, nchunk * NB)
    return np.ascontiguousarray(biasrows, dtype=np.float32), \
        np.ascontiguousarray(maskt)


def _make_in_maps(x, pos_bias_fwd, pos_bias_bwd, beta, alpha, arange2, mask,
                  past_steps, n_cores=N_CORES):
    B, S, C = x.shape
    assert C == NB and S % CHUNK == 0 and B % (n_cores * BG) == 0
    BL = B // n_cores
    P = int(np.asarray(past_steps))
    assert 1 <= P <= 64, f"past_steps={P} outside supported window"

    wdiag, wprev, wbias = _weights(np.asarray(alpha)[0], np.asarray(beta)[0], P)
    biasrows, maskt = _tables(pos_bias_fwd, pos_bias_bwd, arange2, mask, S)

    common = {
        "wdiag": wdiag, "wprev": wprev, "wbias": wbias,
        "biasrows": biasrows, "maskt": maskt,
    }
    in_maps = []
    for i in range(n_cores):
        xs = np.ascontiguousarray(
            x[i * BL:(i + 1) * BL].transpose(1, 0, 2))     # (S, BL, NB)
        in_maps.append({"x": xs, **common})
    return in_maps, BL


def _run(x, pos_bias_fwd, pos_bias_bwd, beta, alpha, arange2, mask, past_steps,
         repeats=1, mm_dt_name="float32r"):
    B, S, C = x.shape
    in_maps, BL = _make_in_maps(
        x, pos_bias_fwd, pos_bias_bwd, beta, alpha, arange2, mask, past_steps)
    nc = _build_nc(S, BL, repeats, mm_dt_name)
    res = run_bass_kernel_spmd(nc, in_maps, core_ids=list(range(N_CORES)))
    out = np.empty((B, S, C), dtype=np.float32)
    for i in range(N_CORES):
        out[i * BL:(i + 1) * BL] = res.results[i]["y"].transpose(1, 0, 2)
    return out


def kernel(x, pos_bias_fwd, pos_bias_bwd, beta, alpha, arange2, mask,
           past_steps, **_unused):
    x = np.asarray(x, dtype=np.float32)
    return _run(x, pos_bias_fwd, pos_bias_bwd, beta, alpha, arange2, mask,
                past_steps)
